# revision 8
# baseline (speedup 1.0000x reference)
"""Trainium2 Bass kernel for nn_DGCNConv (DGCNN-style GNN with sortpooling).

Strategy (data-parallel over output rows / graphs, 8 cores):
  - Host builds the dense symmetrized adjacency A_T (entries are small
    integers -> EXACT in bf16) and the degree scalings d1 = d_in^-1/2,
    d2 = d_out^-1/2.
  - Each core owns 1024 rows (4 graphs). The big matmuls A_T @ S are row
    sharded: core c computes rows [1024c, 1024c+1024) and needs the full S,
    which is produced by an AllGather each round.
  - Algebra: using symmetry of A_T and commutation of row-scaling with
    right-multiplication:
        cur' = tanh(d .* (A_T @ S) + b),   S = d .* (cur @ W^T)
    so the tiny feature matmul (128x128) happens BEFORE the big adjacency
    matmul, and all scalings are per-partition ops.
  - Precision: the moving operand S is split hi/lo into two bf16 tensors
    (S ~= hi + lo, error ~2^-18) so A_T @ S is fp32-accurate at 2 bf16
    passes, except the "cat" branches of layers 0..2 (they only affect
    output values, not the top-k selection) which run single-pass bf16.
  - Per-graph top-k (k=60) sortpooling is done on-device via rank
    computation (comparison counting with stable tie-break) and a
    permutation-matrix matmul gather.
"""

import numpy as np
import ml_dtypes

# ---- problem constants (hardcoded per contract) ----
N = 8192          # nodes
F = 128           # feature dim
L = 4             # layers
G = 32            # graphs
NPG = 256         # nodes per graph
K = 60            # sortpool k
NC = 8            # cores
P = 128           # partitions
ROWS = N // NC    # 1024 rows per core
IT = ROWS // P    # 8 i-tiles per core
JT = N // P       # 64 j-tiles
D = L * F         # 512 output feature dim
GPC = G // NC     # 4 graphs per core

BF16 = ml_dtypes.bfloat16

_CACHE = {}


def _build_nc():
    """Build + compile the Bass program (shared SPMD binary for all 8 cores)."""
    import concourse.bass as bass
    import concourse.bacc as bacc
    import concourse.mybir as mybir
    import concourse.tile as tile

    dt = mybir.dt
    Alu = mybir.AluOpType
    Act = mybir.ActivationFunctionType
    X = mybir.AxisListType.X
    f32 = dt.float32
    bf16 = dt.bfloat16

    nc = bacc.Bacc(
        "TRN2",
        target_bir_lowering=False,
        debug=False,
        enable_asserts=False,
        num_devices=NC,
    )

    # ---------------- I/O ----------------
    at_in = nc.dram_tensor("at", [IT, P, JT * P], bf16, kind="ExternalInput")
    xt_in = nc.dram_tensor("xt", [P, ROWS], f32, kind="ExternalInput")
    w1t_in = nc.dram_tensor("w1t", [L, P, P], f32, kind="ExternalInput")
    w2t_in = nc.dram_tensor("w2t", [L, P, P], f32, kind="ExternalInput")
    w3t_in = nc.dram_tensor("w3t", [L, P, P], f32, kind="ExternalInput")
    b1_in = nc.dram_tensor("b1bc", [L, P, P], f32, kind="ExternalInput")
    b2_in = nc.dram_tensor("b2bc", [L, P, P], f32, kind="ExternalInput")
    b3_in = nc.dram_tensor("b3bc", [L, P, P], f32, kind="ExternalInput")
    d1_in = nc.dram_tensor("d1m", [P, IT], f32, kind="ExternalInput")
    d2_in = nc.dram_tensor("d2m", [P, IT], f32, kind="ExternalInput")
    ident_in = nc.dram_tensor("ident", [P, P], f32, kind="ExternalInput")
    iota_in = nc.dram_tensor("iotar", [P, P], f32, kind="ExternalInput")
    ltc_in = nc.dram_tensor("ltc", [2, P, NPG], f32, kind="ExternalInput")
    ones_in = nc.dram_tensor("ones1", [1, P], f32, kind="ExternalInput")
    out_t = nc.dram_tensor("out", [GPC * K, D], f32, kind="ExternalOutput")

    # stage widths per round: r0: [S1,S2]x(hi,lo) ; r1-3: [S3,S4,S1,S2]hi+[S1,S2]lo
    # r4: [S3,S4]x(hi,lo)
    SR = [512, 768, 768, 768, 512]
    WHI = [256, 512, 512, 512, 256]   # hi width (also psum width)
    WLO = [256, 256, 256, 256, 256]   # lo width; lo targets psum cols [WHI-WLO:WHI]

    rg = [list(range(NC))]

    with tile.TileContext(nc) as tc:
        with (
            tc.tile_pool(name="const", bufs=1) as cp,
            tc.tile_pool(name="dram", bufs=1, space="DRAM") as dp,
            tc.tile_pool(name="sgath", bufs=64) as sgp,
            tc.tile_pool(name="atp", bufs=3) as atp,
            tc.tile_pool(name="stgp", bufs=3) as stgp,
            tc.tile_pool(name="pbig", bufs=2, space="PSUM") as pbig,
            tc.tile_pool(name="psm", bufs=5, space="PSUM") as psm,
            tc.tile_pool(name="tmp", bufs=3) as tmpp,
            tc.tile_pool(name="curp", bufs=3) as curp,
        ):
            # ---- persistent constants / state ----
            def const_tile(shape, nm):
                return cp.tile(shape, f32, name=nm, tag=nm)

            ident = const_tile([P, P], "ident_sb")
            iotar = const_tile([P, P], "iotar_sb")
            ltc = const_tile([P, 2 * NPG], "ltc_sb")
            ones1 = const_tile([1, P], "ones1_sb")
            xt = const_tile([P, ROWS], "xt_sb")
            w1t = const_tile([P, L * P], "w1t_sb")
            w2t = const_tile([P, L * P], "w2t_sb")
            w3t = const_tile([P, L * P], "w3t_sb")
            b1bc = const_tile([P, L * P], "b1bc_sb")
            b2bc = const_tile([P, L * P], "b2bc_sb")
            b3bc = const_tile([P, L * P], "b3bc_sb")
            d1m = const_tile([P, IT], "d1m_sb")
            d2m = const_tile([P, IT], "d2m_sb")
            feat = const_tile([P, IT * D], "feat_sb")

            nc.sync.dma_start(ident[:], ident_in[:])
            nc.sync.dma_start(iotar[:], iota_in[:])
            nc.sync.dma_start(ltc[:, 0:NPG], ltc_in[0])
            nc.sync.dma_start(ltc[:, NPG:2 * NPG], ltc_in[1])
            nc.sync.dma_start(ones1[:], ones_in[:])
            nc.sync.dma_start(xt[:], xt_in[:])
            for lv in range(L):
                sl = slice(lv * P, (lv + 1) * P)
                nc.sync.dma_start(w1t[:, sl], w1t_in[lv])
                nc.sync.dma_start(w2t[:, sl], w2t_in[lv])
                nc.sync.dma_start(w3t[:, sl], w3t_in[lv])
                nc.sync.dma_start(b1bc[:, sl], b1_in[lv])
                nc.sync.dma_start(b2bc[:, sl], b2_in[lv])
                nc.sync.dma_start(b3bc[:, sl], b3_in[lv])
            nc.sync.dma_start(d1m[:], d1_in[:])
            nc.sync.dma_start(d2m[:], d2_in[:])

            # DRAM bounce buffers for the AllGathers
            stage_d = []
            gath_d = []
            for r in range(5):
                stage_d.append(dp.tile([ROWS, SR[r]], bf16, name=f"stage{r}",
                                       tag=f"stage{r}"))
                gath_d.append(dp.tile([N, SR[r]], bf16, name=f"gath{r}",
                                      tag=f"gath{r}", addr_space="Shared"))

            def hilo_write(stg, hi_off, lo_off, src_ps, dcol):
                """stg[:, hi] = bf16(d*src); stg[:, lo] = bf16(d*src - hi)."""
                t = tmpp.tile([P, P], f32, tag="hilo_f32")
                nc.vector.tensor_scalar_mul(t[:], src_ps[:], dcol)
                nc.vector.tensor_copy(stg[:, hi_off:hi_off + P], t[:])
                if lo_off is not None:
                    nc.vector.scalar_tensor_tensor(
                        stg[:, lo_off:lo_off + P],
                        stg[:, hi_off:hi_off + P], -1.0, t[:],
                        Alu.mult, Alu.add)

            # ---------------- stage 0: S1/S2 from x ----------------
            for it in range(IT):
                xsl = xt[:, it * P:(it + 1) * P]
                r1 = psm.tile([P, P], f32, tag="smallps")
                nc.tensor.matmul(r1[:], xsl, w1t[:, 0:P], start=True, stop=True)
                r2 = psm.tile([P, P], f32, tag="smallps")
                nc.tensor.matmul(r2[:], xsl, w2t[:, 0:P], start=True, stop=True)
                stg = stgp.tile([P, SR[0]], bf16, tag="stg")
                hilo_write(stg, 0, 256, r1, d1m[:, it:it + 1])
                hilo_write(stg, P, 256 + P, r2, d2m[:, it:it + 1])
                nc.sync.dma_start(stage_d[0][it * P:(it + 1) * P, :], stg[:])

            def combine(dst, a, b):
                """dst = 0.5*(0.1*a + 0.9*b), matching reference rounding."""
                t1 = tmpp.tile([P, P], f32, tag="comb1")
                nc.vector.tensor_scalar_mul(t1[:], a[:], 0.1)
                t2 = tmpp.tile([P, P], f32, tag="comb2")
                nc.vector.scalar_tensor_tensor(t2[:], b[:], 0.9, t1[:],
                                               Alu.mult, Alu.add)
                nc.vector.tensor_scalar_mul(dst, t2[:], 0.5)

            # ---------------- rounds ----------------
            for r in range(5):
                # All-gather this round's S, then load to SBUF
                nc.gpsimd.collective_compute(
                    "AllGather", Alu.bypass, replica_groups=rg,
                    ins=[stage_d[r].opt()], outs=[gath_d[r].opt()])
                s_tiles = []
                for jt in range(JT):
                    st = sgp.tile([P, SR[r]], bf16, tag="s")
                    nc.sync.dma_start(st[:], gath_d[r][jt * P:(jt + 1) * P, :])
                    s_tiles.append(st)

                whi, wlo = WHI[r], WLO[r]
                lo_ps_off = whi - wlo
                has_cats = r >= 1
                has_curs = r <= 3
                lv_cat = r - 1   # W3/b3 layer for the cats of this round
                lv_cur = r       # W1/W2/b1/b2 layer for the cur update

                for it in range(IT):
                    st0 = atp.tile([P, JT * P // 2], bf16, tag="at")
                    nc.sync.dma_start(st0[:], at_in[it, :, 0:JT * P // 2])
                    st1 = atp.tile([P, JT * P // 2], bf16, tag="at")
                    nc.sync.dma_start(st1[:], at_in[it, :, JT * P // 2:JT * P])

                    ps = pbig.tile([P, whi], f32, tag="pbig")
                    for jt in range(JT):
                        strip = st0 if jt < 32 else st1
                        lo = (jt % 32) * P
                        lhsT = strip[:, lo:lo + P]
                        nc.tensor.matmul(ps[:, 0:whi], lhsT,
                                         s_tiles[jt][:, 0:whi],
                                         start=(jt == 0), stop=False)
                        nc.tensor.matmul(ps[:, lo_ps_off:whi], lhsT,
                                         s_tiles[jt][:, whi:whi + wlo],
                                         start=False, stop=(jt == JT - 1))

                    d1c = d1m[:, it:it + 1]
                    d2c = d2m[:, it:it + 1]

                    def act_group(goff, dcol, bias_sb, lv):
                        pre = tmpp.tile([P, P], f32, tag="pre")
                        nc.vector.scalar_tensor_tensor(
                            pre[:], ps[:, goff:goff + P], dcol,
                            bias_sb[:, lv * P:(lv + 1) * P],
                            Alu.mult, Alu.add)
                        o = curp.tile([P, P], f32, tag="actout")
                        nc.scalar.activation(o[:], pre[:], Act.Tanh)
                        return o

                    goff = 0
                    if has_cats:
                        c3 = act_group(0, d1c, b3bc, lv_cat)
                        c4 = act_group(P, d2c, b3bc, lv_cat)
                        combine(feat[:, it * D + lv_cat * P:
                                      it * D + (lv_cat + 1) * P], c3, c4)
                        goff = 2 * P

                    if has_curs:
                        cur1 = act_group(goff, d1c, b1bc, lv_cur)
                        cur2 = act_group(goff + P, d2c, b2bc, lv_cur)

                        # transposes for the next small matmuls
                        t1p = psm.tile([P, P], f32, tag="smallps")
                        nc.tensor.transpose(t1p[:], cur1[:], ident[:])
                        cur1T = curp.tile([P, P], f32, tag="curT")
                        nc.vector.tensor_copy(cur1T[:], t1p[:])
                        t2p = psm.tile([P, P], f32, tag="smallps")
                        nc.tensor.transpose(t2p[:], cur2[:], ident[:])
                        cur2T = curp.tile([P, P], f32, tag="curT")
                        nc.vector.tensor_copy(cur2T[:], t2p[:])
                        outT = curp.tile([P, P], f32, tag="outT")
                        combine(outT[:], cur1T, cur2T)

                        # small matmuls -> stage r+1
                        rn = r + 1
                        stg = stgp.tile([P, SR[rn]], bf16, tag="stg")
                        r3 = psm.tile([P, P], f32, tag="smallps")
                        nc.tensor.matmul(
                            r3[:], outT[:],
                            w3t[:, lv_cur * P:(lv_cur + 1) * P],
                            start=True, stop=True)
                        if rn == 4:
                            # [S3h S4h | S3l S4l]
                            hilo_write(stg, 0, 2 * P, r3, d1c)
                            hilo_write(stg, P, 3 * P, r3, d2c)
                        else:
                            # [S3h S4h S1h S2h | S1l S2l]
                            hilo_write(stg, 0, None, r3, d1c)
                            hilo_write(stg, P, None, r3, d2c)
                            r1n = psm.tile([P, P], f32, tag="smallps")
                            nc.tensor.matmul(
                                r1n[:], cur1T[:],
                                w1t[:, (lv_cur + 1) * P:(lv_cur + 2) * P],
                                start=True, stop=True)
                            hilo_write(stg, 2 * P, 4 * P, r1n, d1c)
                            r2n = psm.tile([P, P], f32, tag="smallps")
                            nc.tensor.matmul(
                                r2n[:], cur2T[:],
                                w2t[:, (lv_cur + 1) * P:(lv_cur + 2) * P],
                                start=True, stop=True)
                            hilo_write(stg, 3 * P, 5 * P, r2n, d2c)
                        nc.sync.dma_start(
                            stage_d[rn][it * P:(it + 1) * P, :], stg[:])

            # ---------------- sortpooling ----------------
            for g in range(GPC):
                # channel values of the two node tiles of this graph
                repl = []
                for tf in range(2):
                    it = 2 * g + tf
                    col = feat[:, it * D + D - 1:it * D + D]   # [P, 1]
                    vtp = psm.tile([1, P], f32, tag="smallps")
                    nc.tensor.transpose(vtp[:], col, ident[:])
                    vrow = tmpp.tile([1, P], f32, tag="vrow")
                    nc.vector.tensor_copy(vrow[:], vtp[:])
                    rp = psm.tile([P, P], f32, tag="smallps")
                    nc.tensor.matmul(rp[:], ones1[:], vrow[:],
                                     start=True, stop=True)
                    repl.append(rp)
                poolps = pbig.tile([P, D], f32, tag="pbig")
                for tp in range(2):
                    it = 2 * g + tp
                    vcol = feat[:, it * D + D - 1:it * D + D]
                    Ct = tmpp.tile([P, NPG], f32, tag="Ct")
                    Et = tmpp.tile([P, NPG], f32, tag="Et")
                    for tf in range(2):
                        sl = slice(tf * P, (tf + 1) * P)
                        nc.vector.tensor_scalar(Ct[:, sl], repl[tf][:], vcol,
                                                None, Alu.is_gt)
                        nc.vector.tensor_scalar(Et[:, sl], repl[tf][:], vcol,
                                                None, Alu.is_equal)
                    nc.vector.tensor_tensor(
                        Et[:], Et[:], ltc[:, tp * NPG:(tp + 1) * NPG],
                        Alu.mult)
                    nc.vector.tensor_tensor(Ct[:], Ct[:], Et[:], Alu.add)
                    ranks = tmpp.tile([P, 1], f32, tag="ranks")
                    nc.vector.reduce_sum(ranks[:], Ct[:], axis=X)
                    perm = curp.tile([P, P], f32, tag="perm")
                    nc.vector.tensor_scalar(perm[:], iotar[:], ranks[:],
                                            None, Alu.is_equal)
                    nc.tensor.matmul(poolps[:], perm[:],
                                     feat[:, it * D:(it + 1) * D],
                                     start=(tp == 0), stop=(tp == 1))
                osb = tmpp.tile([P, D], f32, tag="osb")
                nc.vector.tensor_copy(osb[0:K, :], poolps[0:K, :])
                nc.sync.dma_start(out_t[g * K:(g + 1) * K, :], osb[0:K, :])

    nc.compile()
    return nc


def _host_prep(x, edge_index):
    """Build A_T, degree scalings and all per-core constant arrays."""
    ei = np.asarray(edge_index).astype(np.int64)
    x = np.asarray(x).astype(np.float32)

    counts = np.bincount(ei[0] * N + ei[1], minlength=N * N)
    A = counts.reshape(N, N).astype(np.float32)
    del counts
    A[np.arange(N), np.arange(N)] += 1.0
    d1 = (A.sum(axis=0)) ** np.float32(-0.5)   # d_in  (column sums)
    d2 = (A.sum(axis=1)) ** np.float32(-0.5)   # d_out (row sums)
    AT = (A + A.T).astype(BF16)                # entries are small ints: exact
    del A
    return x, AT, d1.astype(np.float32), d2.astype(np.float32)


def _core_inputs(x, AT, d1, d2, W1, b1, W2, b2, W3, b3):
    """Returns list of per-core in_maps."""
    w1t = np.ascontiguousarray(np.asarray(W1).astype(np.float32).transpose(0, 2, 1))
    w2t = np.ascontiguousarray(np.asarray(W2).astype(np.float32).transpose(0, 2, 1))
    w3t = np.ascontiguousarray(np.asarray(W3).astype(np.float32).transpose(0, 2, 1))
    b1bc = np.ascontiguousarray(
        np.broadcast_to(np.asarray(b1, np.float32)[:, None, :], (L, P, P)))
    b2bc = np.ascontiguousarray(
        np.broadcast_to(np.asarray(b2, np.float32)[:, None, :], (L, P, P)))
    b3bc = np.ascontiguousarray(
        np.broadcast_to(np.asarray(b3, np.float32)[:, None, :], (L, P, P)))
    ident = np.eye(P, dtype=np.float32)
    iotar = np.broadcast_to(np.arange(P, dtype=np.float32)[None, :], (P, P))
    iotar = np.ascontiguousarray(iotar)
    fidx = np.arange(NPG)[None, :]
    ltc = np.stack([
        (fidx < (t * P + np.arange(P)[:, None])).astype(np.float32)
        for t in range(2)])
    ltc = np.ascontiguousarray(ltc)
    ones1 = np.ones((1, P), dtype=np.float32)

    in_maps = []
    for c in range(NC):
        off = c * ROWS
        ats = AT[:, off:off + ROWS]  # [8192, 1024] bf16
        at = np.ascontiguousarray(
            ats.reshape(JT, P, IT, P).transpose(2, 1, 0, 3).reshape(IT, P, JT * P))
        xt = np.ascontiguousarray(x[off:off + ROWS].T)
        d1m = np.ascontiguousarray(d1[off:off + ROWS].reshape(IT, P).T)
        d2m = np.ascontiguousarray(d2[off:off + ROWS].reshape(IT, P).T)
        in_maps.append({
            "at": at, "xt": xt,
            "w1t": w1t, "w2t": w2t, "w3t": w3t,
            "b1bc": b1bc, "b2bc": b2bc, "b3bc": b3bc,
            "d1m": d1m, "d2m": d2m,
            "ident": ident, "iotar": iotar, "ltc": ltc, "ones1": ones1,
        })
    return in_maps


def _get_runner():
    """Build (once) a cached jitted SPMD callable for the compiled program."""
    if "runner" in _CACHE:
        return _CACHE["runner"]
    import jax
    import concourse.mybir as mybir
    from concourse.bass2jax import (
        _bass_exec_p, install_neuronx_cc_hook, partition_id_tensor)
    from jax.experimental.shard_map import shard_map
    from jax.sharding import Mesh, PartitionSpec, NamedSharding

    install_neuronx_cc_hook()
    if "nc" not in _CACHE:
        _CACHE["nc"] = _build_nc()
    nc = _CACHE["nc"]

    part_name = (nc.partition_id_tensor.name
                 if nc.partition_id_tensor else None)
    in_names, out_names, out_avals, zero_outs = [], [], [], []
    for alloc in nc.m.functions[0].allocations:
        if not isinstance(alloc, mybir.MemoryLocationSet):
            continue
        name = alloc.memorylocations[0].name
        if alloc.kind == "ExternalInput":
            if name != part_name:
                in_names.append(name)
        elif alloc.kind == "ExternalOutput":
            out_names.append(name)
            shape = tuple(alloc.tensor_shape)
            dtype = mybir.dt.np(alloc.dtype)
            out_avals.append(jax.core.ShapedArray(shape, dtype))
            zero_outs.append((shape, dtype))
    n_params = len(in_names)
    n_outs = len(out_names)
    all_names = in_names + out_names
    if part_name is not None:
        all_names = all_names + [part_name]

    def _body(*args):
        operands = list(args)
        if part_name is not None:
            operands.append(partition_id_tensor())
        outs = _bass_exec_p.bind(
            *operands,
            out_avals=tuple(out_avals),
            in_names=tuple(all_names),
            out_names=tuple(out_names),
            lowering_input_output_aliases=(),
            sim_require_finite=True,
            sim_require_nnan=True,
            nc=nc,
        )
        return tuple(outs)

    devices = jax.devices()[:NC]
    mesh = Mesh(np.asarray(devices), ("core",))
    spec = PartitionSpec("core")
    donate = tuple(range(n_params, n_params + n_outs))
    fn = jax.jit(
        shard_map(_body, mesh=mesh, in_specs=(spec,) * (n_params + n_outs),
                  out_specs=(spec,) * n_outs, check_rep=False),
        donate_argnums=donate, keep_unused=True)
    sharding = NamedSharding(mesh, spec)
    runner = {
        "fn": fn, "in_names": in_names, "out_names": out_names,
        "zero_outs": zero_outs, "sharding": sharding, "jax": jax,
    }
    _CACHE["runner"] = runner
    return runner


def _prep_device_inputs(inputs):
    """Host prep + upload per-core inputs to the devices (cached by id of x)."""
    r = _get_runner()
    jax = r["jax"]
    x, AT, d1, d2 = _host_prep(inputs["x"], inputs["edge_index"])
    in_maps = _core_inputs(x, AT, d1, d2,
                           inputs["W1"], inputs["b1"],
                           inputs["W2"], inputs["b2"],
                           inputs["W3"], inputs["b3"])
    concat = [np.concatenate([in_maps[c][nm] for c in range(NC)], axis=0)
              for nm in r["in_names"]]
    dev_in = [jax.device_put(a, r["sharding"]) for a in concat]
    for a in dev_in:
        a.block_until_ready()
    return dev_in


def _exec(dev_in):
    r = _get_runner()
    jax = r["jax"]
    zeros = [jax.device_put(np.zeros((NC * s[0], *s[1:]), d), r["sharding"])
             for s, d in r["zero_outs"]]
    for z in zeros:
        z.block_until_ready()
    import time
    t0 = time.perf_counter()
    outs = r["fn"](*dev_in, *zeros)
    outs = jax.block_until_ready(outs)
    t1 = time.perf_counter()
    return outs, (t1 - t0)


def _run(inputs, reps=1):
    r = _get_runner()
    dev_in = _prep_device_inputs(inputs)
    times = []
    outs = None
    for _ in range(max(1, reps)):
        outs, dt = _exec(dev_in)
        times.append(dt)
    arr = np.asarray(outs[r["out_names"].index("out")])
    pooled = arr.reshape(NC, GPC * K, D).reshape(G, K, D)
    return np.ascontiguousarray(pooled.astype(np.float32)), times


def kernel(**inputs) -> np.ndarray:
    out, _ = _run(inputs, reps=1)
    return out
